# revision 8
# baseline (speedup 1.0000x reference)
"""Trainium2 Bass kernel for nn_DownSample (sparse-conv GNN downsample stack).

Strategy (8 NeuronCores, SPMD):
  - Destinations sharded across cores; gather tables replicated via native
    AllGather collectives between layers (tables live in DRAM, bf16).
  - All sparse-conv gathers are SWDGE indirect DMAs (int32 row indices,
    batched thousands of rows per instruction).
  - Compute in transposed [C, dst] orientation: gathered im2col rows are
    PE-transposed in 128-chunks, matmul-accumulated in PSUM against
    reshaped weights, bias+activation on the scalar engine, transposed
    back to row-major on write-out.
  - Host does index preprocessing only (shard remap, per-instruction
    int32 index layouts); all feature compute happens on device.
"""
import sys
import numpy as np

sys.path.insert(0, "/opt/trn_rl_repo")

import ml_dtypes
import concourse.bass as bass
import concourse.bacc as bacc
import concourse.tile as tile
import concourse.mybir as mybir
from concourse.bass_utils import run_bass_kernel_spmd
from concourse.masks import make_identity

P = 128
NCORES = 8
N_IN = 200_000
N_OUT = 50_000
IC0, OC0, OC1 = 3, 64, 64
IC0P = 4                        # feats channels padded for DMA alignment
K3, K2 = 27, 8

SH0 = N_IN // NCORES            # 25000 conv0 dsts per core
SH0P = 25088                    # padded to 49*512
SH1 = N_OUT // NCORES           # 6250
SH1P = 6272                     # padded to 49*128
NF0 = SH0P * NCORES             # 200704 padded h0 table rows
NF1 = SH1P * NCORES             # 50176 padded 50k-table rows

BF = mybir.dt.bfloat16
F32 = mybir.dt.float32
I32 = mybir.dt.int32
GROUP = 512                     # dsts per gather instruction (4 subtiles)


def _groups(total):
    """Yield (base, size) dst groups of up to GROUP, sizes multiple of 128."""
    b = 0
    while b < total:
        g = min(GROUP, total - b)
        yield b, g
        b += g


def _pack_idx(nbr, K):
    """[N, K] int32 -> per-group [128, (g//128)*K] layouts, concatenated flat."""
    out = []
    N = nbr.shape[0]
    for b, g in _groups(N):
        sub = nbr[b : b + g].reshape(g // P, P, K)      # [s, p, K]
        out.append(np.ascontiguousarray(sub.transpose(1, 0, 2)).reshape(P, -1))
    return out


def _remap(idx, sh, shp):
    return (idx // sh) * shp + (idx % sh)


def build_module():
    nc = bacc.Bacc("TRN2", target_bir_lowering=False, debug=False,
                   num_devices=NCORES)

    # ---- I/O ----
    feats = nc.dram_tensor("feats", [N_IN, IC0P], BF, kind="ExternalInput").ap()
    idx0 = nc.dram_tensor("idx0", [SH0P // P // 4, P, 4 * K3], I32, kind="ExternalInput").ap()
    idxd = nc.dram_tensor("idxd", [SH1P // P, P, K2], I32, kind="ExternalInput").ap()
    idxb = nc.dram_tensor("idxb", [SH1P // P, P, K3], I32, kind="ExternalInput").ap()
    w0 = nc.dram_tensor("w0", [K3 * IC0P, OC0], BF, kind="ExternalInput").ap()
    wd = nc.dram_tensor("wd", [P, 4 * OC1], BF, kind="ExternalInput").ap()
    wa = [nc.dram_tensor(f"wa{b}", [P, 14 * 16], BF, kind="ExternalInput").ap() for b in range(3)]
    wc = [nc.dram_tensor(f"wc{b}", [P, 7 * 48], BF, kind="ExternalInput").ap() for b in range(3)]
    w10 = [nc.dram_tensor(f"w10_{b}", [OC1, 16], BF, kind="ExternalInput").ap() for b in range(3)]
    w12 = [nc.dram_tensor(f"w12_{b}", [16, 32], BF, kind="ExternalInput").ap() for b in range(3)]
    # biases stacked per-partition [128, n]: col layout documented in kernel()
    biases = nc.dram_tensor("biases", [P, 20], F32, kind="ExternalInput").ap()
    y = nc.dram_tensor("y", [SH1P, OC1], F32, kind="ExternalOutput").ap()
    import os
    dbg = os.environ.get("KDBG", "") == "1"
    stage = int(os.environ.get("KSTAGE", "9"))
    if dbg:
        yd_h0 = nc.dram_tensor("yd_h0", [SH0P, OC0], BF, kind="ExternalOutput").ap()
        yd_f0 = nc.dram_tensor("yd_f0", [SH1P, OC1], BF, kind="ExternalOutput").ap()
        yd_hh = nc.dram_tensor("yd_hh", [SH1P, 32], BF, kind="ExternalOutput").ap()
        yd_f1 = nc.dram_tensor("yd_f1", [SH1P, OC1], BF, kind="ExternalOutput").ap()

    # ---- internal DRAM tables ----
    h0_sh = nc.dram_tensor("h0_sh", [SH0P, OC0], BF, kind="Internal").ap()
    h0_full = nc.dram_tensor("h0_full", [NF0, OC0], BF, kind="Internal",
                             addr_space="Shared").ap()
    f_sh = [nc.dram_tensor(f"f_sh{i}", [SH1P, OC1], BF, kind="Internal").ap()
            for i in range(3)]  # down0 out, block1 out, block2 out
    f_full = [nc.dram_tensor(f"f_full{i}", [NF1, OC1], BF, kind="Internal",
                             addr_space="Shared").ap() for i in range(3)]
    hh_sh = [nc.dram_tensor(f"hh_sh{b}", [SH1P, 32], BF, kind="Internal").ap()
             for b in range(3)]
    hh_full = [nc.dram_tensor(f"hh_full{b}", [NF1, 32], BF, kind="Internal",
                              addr_space="Shared").ap() for b in range(3)]

    RG = [list(range(NCORES))]

    with tile.TileContext(nc) as tc:
        with (
            tc.tile_pool(name="const", bufs=1) as cpool,
            tc.tile_pool(name="idx", bufs=1) as xpool,
            tc.tile_pool(name="gat", bufs=2) as gpool,
            tc.tile_pool(name="work", bufs=3) as wpool,
            tc.tile_pool(name="outp", bufs=3) as opool,
            tc.tile_pool(name="pt", bufs=2, space="PSUM") as pt,
            tc.tile_pool(name="pacc", bufs=2, space="PSUM") as pacc,
            tc.tile_pool(name="pout", bufs=2, space="PSUM") as pout,
        ):
            ident = cpool.tile([P, P], BF, tag="ident")
            make_identity(nc, ident[:])

            # preload weights
            w0_t = cpool.tile([K3 * IC0P, OC0], BF, tag="w0")
            nc.sync.dma_start(out=w0_t[:], in_=w0[:])
            wd_t = cpool.tile([P, 4 * OC1], BF, tag="wd")
            nc.sync.dma_start(out=wd_t[:], in_=wd[:])
            wa_t, wc_t, w10_t, w12_t = [], [], [], []
            for b in range(3):
                t = cpool.tile([P, 14 * 16], BF, tag=f"wa{b}")
                nc.sync.dma_start(out=t[:], in_=wa[b][:]); wa_t.append(t)
                t = cpool.tile([P, 7 * 48], BF, tag=f"wc{b}")
                nc.sync.dma_start(out=t[:], in_=wc[b][:]); wc_t.append(t)
                t = cpool.tile([OC1, 16], BF, tag=f"w10{b}")
                nc.sync.dma_start(out=t[:], in_=w10[b][:]); w10_t.append(t)
                t = cpool.tile([16, 32], BF, tag=f"w12{b}")
                nc.sync.dma_start(out=t[:], in_=w12[b][:]); w12_t.append(t)
            bias_t = cpool.tile([P, 20], F32, tag="bias")
            nc.sync.dma_start(out=bias_t[:], in_=biases[:])

            # preload block idx tiles (reused 6x)
            idxb_t = []
            for s in range(SH1P // P):
                t = xpool.tile([P, K3], I32, tag=f"ib{s}")
                nc.sync.dma_start(out=t[:], in_=idxb[s])
                idxb_t.append(t)

            def cp_engine(i):
                return nc.vector if (i % 2 == 0) else nc.scalar

            def copy(i, out, in_):
                if i % 2 == 0:
                    nc.vector.tensor_copy(out=out, in_=in_)
                else:
                    nc.scalar.copy(out=out, in_=in_)

            # ---------- generic transposed sconv over one dst group ----------
            def sconv_group(g_tile, gsz, K, Cin, w_tile, w_cols, Cout, ctr_chunks,
                            act_fn, bias_ap, out_cb, extra_mm=None):
                """g_tile: [128, (gsz//128)*K*Cin] gathered rows (bf16).
                ctr_chunks: list of (rows, lhsT col slice base) chunk sizes.
                out_cb(sub, act_sbuf [Cout, 128]) consumes each subtile result."""
                nsub = gsz // P
                ctr = K * Cin
                for s in range(nsub):
                    aT = wpool.tile([P, ((ctr + P - 1) // P) * P], BF, tag="aT")
                    nchunk = (ctr + P - 1) // P
                    for q in range(nchunk):
                        rows = min(P, ctr - q * P)
                        src = g_tile[:, s * ctr + q * P : s * ctr + q * P + rows]
                        pT = pt.tile([P, P], BF, tag="pT")
                        nc.tensor.transpose(out=pT[:rows, :], in_=src, identity=ident[:])
                        copy(q, aT[:rows, q * P : q * P + P], pT[:rows, :])
                    pacc_t = pacc.tile([Cout, P], F32, tag="pacc")
                    for q in range(nchunk):
                        rows = min(P, ctr - q * P)
                        nc.tensor.matmul(
                            out=pacc_t[:],
                            lhsT=w_tile[:rows, q * w_cols : q * w_cols + Cout],
                            rhs=aT[:rows, q * P : q * P + P],
                            start=(q == 0), stop=(q == nchunk - 1),
                        )
                    out_cb(s, pacc_t)

            # out helper: transpose [C,128] slices into row tile and DMA out
            def emit_rows(sub_results, dst_dram, gbase, gsz, rowC, resid_rows=None,
                          out_f32=False):
                # sub_results: list per subtile of list of (sbuf [c,128], c)
                nsub = gsz // P
                for s in range(nsub):
                    orow = opool.tile([P, rowC], F32 if out_f32 else BF, tag="orow")
                    col = 0
                    for (t_sb, c) in sub_results[s]:
                        po = pout.tile([P, rowC], BF, tag="po")
                        nc.tensor.transpose(out=po[:, :c], in_=t_sb[:c, :],
                                            identity=ident[:c, :c])
                        if resid_rows is not None:
                            nc.vector.tensor_add(
                                out=orow[:, col : col + c],
                                in0=po[:, :c],
                                in1=resid_rows[s][:, col : col + c],
                            )
                        else:
                            copy(col, orow[:, col : col + c], po[:, :c])
                        col += c
                    nc.sync.dma_start(
                        out=dst_dram[gbase + s * P : gbase + s * P + P, :],
                        in_=orow[:],
                    )

            # ================= conv0 =================
            for gi, (gb, gsz) in enumerate(_groups(SH0P)):
                it = xpool.tile([P, 4 * K3], I32, tag="i0")
                nc.sync.dma_start(out=it[:], in_=idx0[gb // GROUP])
                g = gpool.tile([P, (gsz // P) * K3 * IC0P], BF, tag="g")
                for s in range(gsz // P):
                    for k in range(K3):
                        nc.gpsimd.indirect_dma_start(
                            out=g[:, (s * K3 + k) * IC0P : (s * K3 + k + 1) * IC0P],
                            out_offset=None, in_=feats[:],
                            in_offset=bass.IndirectOffsetOnAxis(
                                ap=it[:, s * K3 + k : s * K3 + k + 1], axis=0),
                        )
                subres = []
                def c0_out(s, pacc_t, _sub=subres):
                    h = wpool.tile([OC0, P], BF, tag="h0t")
                    nc.scalar.activation(h[:], pacc_t[:],
                                         mybir.ActivationFunctionType.Relu,
                                         bias=bias_t[:OC0, 0:1])
                    _sub.append([(h, OC0)])
                sconv_group(g, gsz, K3, IC0P, w0_t, OC0, OC0, None,
                            None, None, c0_out)
                emit_rows(subres, h0_sh, gb, gsz, OC0)

            # ================= down0 =================
            if stage >= 2:
                nc.gpsimd.collective_compute(
                    "AllGather", mybir.AluOpType.bypass, replica_groups=RG,
                    ins=[h0_sh[:]], outs=[h0_full[:]])
            for gb, gsz in (_groups(SH1P) if stage >= 2 else []):
                it = xpool.tile([P, K2 * 4], I32, tag="id")
                nc.sync.dma_start(
                    out=it[:, : (gsz // P) * K2].rearrange(
                        "p (s k) -> p s k", k=K2),
                    in_=idxd[gb // P : gb // P + gsz // P].rearrange(
                        "s p k -> p s k"))
                g = gpool.tile([P, (gsz // P) * K2 * OC0], BF, tag="g")
                for s in range(gsz // P):
                    for k in range(K2):
                        nc.gpsimd.indirect_dma_start(
                            out=g[:, (s * K2 + k) * OC0 : (s * K2 + k + 1) * OC0],
                            out_offset=None, in_=h0_full[:],
                            in_offset=bass.IndirectOffsetOnAxis(
                                ap=it[:, s * K2 + k : s * K2 + k + 1], axis=0),
                        )
                subres = []
                def d0_out(s, pacc_t, _sub=subres):
                    h = wpool.tile([OC1, P], BF, tag="h1t")
                    nc.scalar.activation(h[:], pacc_t[:],
                                         mybir.ActivationFunctionType.Relu,
                                         bias=bias_t[:OC1, 1:2])
                    _sub.append([(h, OC1)])
                sconv_group(g, gsz, K2, OC0, wd_t, OC1, OC1, None,
                            None, None, d0_out)
                emit_rows(subres, f_sh[0], gb, gsz, OC1)

            if stage >= 3:
                nc.gpsimd.collective_compute(
                    "AllGather", mybir.AluOpType.bypass, replica_groups=RG,
                    ins=[f_sh[0][:]], outs=[f_full[0][:]])

            # ================= blocks =================
            nblocks = 0 if stage < 3 else (1 if stage == 3 else 3)
            for b in range(nblocks):
                src_full = f_full[b]
                # my shard's f rows for w10 + residual: rows [cbase, cbase+SH1P)
                # cbase differs per core -> host passes idx? simpler: shard rows
                # were just written by this core pre-AG: read from f_sh of layer b
                src_sh = f_sh[b]

                # ---- stage A: w00 sconv + w10 branch -> hh ----
                for gb, gsz in _groups(SH1P):
                    nsub = gsz // P
                    it_cols = [idxb_t[gb // P + s] for s in range(nsub)]
                    # build one combined idx tile view per group: gather per subtile
                    g = gpool.tile([P, nsub * K3 * OC1], BF, tag="g")
                    for s in range(nsub):
                        for k in range(K3):
                            nc.gpsimd.indirect_dma_start(
                                out=g[:, (s * K3 + k) * OC1 : (s * K3 + k + 1) * OC1],
                                out_offset=None, in_=src_full[:],
                                in_offset=bass.IndirectOffsetOnAxis(
                                    ap=it_cols[s][:, k : k + 1], axis=0),
                            )
                    subres = []
                    def a_out(s, pacc_t, _sub=subres, _b=b):
                        h00 = wpool.tile([16, P], BF, tag="h00")
                        nc.scalar.activation(h00[:], pacc_t[:16, :],
                                             mybir.ActivationFunctionType.Relu,
                                             bias=bias_t[:16, 2 + 5 * _b : 3 + 5 * _b])
                        _sub.append([(h00, 16)])
                    sconv_group(g, gsz, K3, OC1, wa_t[b], 16, 16, None,
                                None, None, a_out)
                    # w10 branch per subtile
                    for s in range(gsz // P):
                        frow = wpool.tile([P, OC1], BF, tag="frow")
                        nc.sync.dma_start(
                            out=frow[:],
                            in_=src_sh[gb + s * P : gb + s * P + P, :])
                        pf = pt.tile([P, P], BF, tag="pT")
                        nc.tensor.transpose(out=pf[:OC1, :], in_=frow[:, :],
                                            identity=ident[:])
                        fT = wpool.tile([OC1, P], BF, tag="fT")
                        copy(s, fT[:], pf[:OC1, :])
                        pm = pacc.tile([16, P], F32, tag="pacc")
                        nc.tensor.matmul(out=pm[:], lhsT=w10_t[b][:],
                                         rhs=fT[:], start=True, stop=True)
                        h1a = wpool.tile([16, P], BF, tag="h1a")
                        nc.scalar.activation(h1a[:], pm[:],
                                             mybir.ActivationFunctionType.Relu,
                                             bias=bias_t[:16, 3 + 5 * b : 4 + 5 * b])
                        subres[s].append((h1a, 16))
                    emit_rows(subres, hh_sh[b], gb, gsz, 32)

                nc.gpsimd.collective_compute(
                    "AllGather", mybir.AluOpType.bypass, replica_groups=RG,
                    ins=[hh_sh[b][:]], outs=[hh_full[b][:]])

                # ---- stage B: w01|w11 combined sconv + w12 + residual ----
                last = b == 2
                out_dram = y if last else f_sh[b + 1]
                for gb, gsz in _groups(SH1P):
                    nsub = gsz // P
                    g = gpool.tile([P, nsub * K3 * 32], BF, tag="g")
                    for s in range(nsub):
                        for k in range(K3):
                            nc.gpsimd.indirect_dma_start(
                                out=g[:, (s * K3 + k) * 32 : (s * K3 + k + 1) * 32],
                                out_offset=None, in_=hh_full[b][:],
                                in_offset=bass.IndirectOffsetOnAxis(
                                    ap=idxb_t[gb // P + s][:, k : k + 1], axis=0),
                            )
                    subres = []
                    resid = []
                    def b_out(s, pacc_t, _sub=subres, _b=b):
                        o0 = wpool.tile([32, P], BF, tag="o0")
                        nc.scalar.activation(o0[:], pacc_t[:32, :],
                                             mybir.ActivationFunctionType.Identity,
                                             bias=bias_t[:32, 4 + 5 * _b : 5 + 5 * _b])
                        h1c = wpool.tile([16, P], BF, tag="h1c")
                        nc.scalar.activation(h1c[:], pacc_t[32:48, :],
                                             mybir.ActivationFunctionType.Relu,
                                             bias=bias_t[:16, 5 + 5 * _b : 6 + 5 * _b])
                        pm2 = pacc.tile([32, P], F32, tag="pacc2")
                        nc.tensor.matmul(out=pm2[:], lhsT=w12_t[_b][:],
                                         rhs=h1c[:], start=True, stop=True)
                        o1 = wpool.tile([32, P], BF, tag="o1")
                        nc.scalar.activation(o1[:], pm2[:],
                                             mybir.ActivationFunctionType.Identity,
                                             bias=bias_t[:32, 6 + 5 * _b : 7 + 5 * _b])
                        _sub.append([(o0, 32), (o1, 32)])
                    sconv_group(g, gsz, K3, 32, wc_t[b], 48, 48, None,
                                None, None, b_out)
                    for s in range(nsub):
                        frow = wpool.tile([P, OC1], BF, tag="frow")
                        nc.sync.dma_start(
                            out=frow[:],
                            in_=src_sh[gb + s * P : gb + s * P + P, :])
                        resid.append(frow)
                    emit_rows(subres, out_dram, gb, gsz, OC1,
                              resid_rows=resid, out_f32=last)

                if not last:
                    nc.gpsimd.collective_compute(
                        "AllGather", mybir.AluOpType.bypass, replica_groups=RG,
                        ins=[f_sh[b + 1][:]], outs=[f_full[b + 1][:]])

            if stage < 9:
                zt = opool.tile([P, OC1], F32, tag="zt")
                nc.vector.memset(zt[:], 0.0)
                for s in range(SH1P // P):
                    nc.sync.dma_start(out=y[s * P:(s + 1) * P, :], in_=zt[:])
            if dbg:
                nc.sync.dma_start(out=yd_h0[:], in_=h0_sh[:])
                nc.sync.dma_start(out=yd_f0[:], in_=f_sh[0][:])
                nc.sync.dma_start(out=yd_hh[:], in_=hh_sh[0][:])
                nc.sync.dma_start(out=yd_f1[:], in_=f_sh[1][:])

    nc.compile()
    return nc


def _prep_host(feats, params, nbr_in3, nbr_down, nbr_out3):
    """Per-core input dicts."""
    bf = ml_dtypes.bfloat16
    feats_bf = np.zeros((N_IN, IC0P), bf)
    feats_bf[:, :IC0] = feats.astype(bf)

    # weights
    W0_raw = np.asarray(params["conv0_w"], np.float32)  # [27, 3, 64]
    W0 = np.zeros((K3 * IC0P, OC0), np.float32)
    for k in range(K3):
        W0[k * IC0P : k * IC0P + IC0] = W0_raw[k]
    WD = np.asarray(params["down0_w"], np.float32).reshape(K2 * OC0, OC1)
    wd_p = np.zeros((P, 4 * OC1), np.float32)
    for q in range(4):
        wd_p[:, q * OC1 : (q + 1) * OC1] = WD[q * P : (q + 1) * P]
    blocks = params["blocks"]
    wa_p, wc_p, w10_p, w12_p = [], [], [], []
    bias = np.zeros((P, 20), np.float32)
    bias[:OC0, 0] = np.asarray(params["conv0_b"])
    bias[:OC1, 1] = np.asarray(params["down0_b"])
    for b, blk in enumerate(blocks):
        wa = np.asarray(blk["w00"], np.float32).reshape(K3 * OC1, 16)  # 1728x16
        wap = np.zeros((P, 14 * 16), np.float32)
        for q in range(14):
            rows = min(P, 1728 - q * P)
            wap[:rows, q * 16 : q * 16 + 16] = wa[q * P : q * P + rows]
        wa_p.append(wap)
        w01 = np.asarray(blk["w01"], np.float32)  # [27,16,32]
        w11 = np.asarray(blk["w11"], np.float32)  # [27,16,16]
        wcc = np.zeros((K3 * 32, 48), np.float32)  # rows (k, c32)
        for k in range(K3):
            wcc[k * 32 : k * 32 + 16, 0:32] = w01[k]
            wcc[k * 32 + 16 : k * 32 + 32, 32:48] = w11[k]
        wcp = np.zeros((P, 7 * 48), np.float32)
        for q in range(7):
            rows = min(P, 864 - q * P)
            wcp[:rows, q * 48 : q * 48 + 48] = wcc[q * P : q * P + rows]
        wc_p.append(wcp)
        w10_p.append(np.asarray(blk["w10"], np.float32))
        w12_p.append(np.asarray(blk["w12"], np.float32))
        bias[:16, 2 + 5 * b] = np.asarray(blk["b00"])
        bias[:16, 3 + 5 * b] = np.asarray(blk["b10"])
        bias[:32, 4 + 5 * b] = np.asarray(blk["b01"])
        bias[:16, 5 + 5 * b] = np.asarray(blk["b11"])
        bias[:32, 6 + 5 * b] = np.asarray(blk["b12"])

    nbr_down_r = _remap(nbr_down.astype(np.int64), SH0, SH0P).astype(np.int32)
    nbr_out3_r = _remap(nbr_out3.astype(np.int64), SH1, SH1P).astype(np.int32)

    in_maps = []
    for c in range(NCORES):
        # conv0 idx: [49, 128, 4*27]
        i0 = np.zeros((SH0P // P // 4, P, 4 * K3), np.int32)
        base = c * SH0
        n = SH0  # real dsts
        dst = np.arange(SH0P)
        src = np.minimum(dst, n - 1) + base
        nb = nbr_in3[src]  # [SH0P, 27]
        nb = nb.reshape(-1, 4, P, K3).transpose(0, 2, 1, 3).reshape(-1, P, 4 * K3)
        i0[:] = nb
        # down idx [49, 128, 8]
        bd = c * SH1
        dstd = np.arange(SH1P)
        srcd = np.minimum(dstd, SH1 - 1) + bd
        idd = nbr_down_r[srcd].reshape(-1, P, K2).astype(np.int32)
        # block idx [49, 128, 27]
        idb = nbr_out3_r[srcd].reshape(-1, P, K3).astype(np.int32)
        m = {
            "feats": feats_bf,
            "idx0": i0.astype(np.int32),
            "idxd": idd,
            "idxb": idb,
            "w0": W0.astype(bf),
            "wd": wd_p.astype(bf),
            "biases": bias.astype(np.float32),
        }
        for b in range(3):
            m[f"wa{b}"] = wa_p[b].astype(bf)
            m[f"wc{b}"] = wc_p[b].astype(bf)
            m[f"w10_{b}"] = w10_p[b].astype(bf)
            m[f"w12_{b}"] = w12_p[b].astype(bf)
        in_maps.append(m)
    return in_maps


_NC_CACHE = {}


def kernel(feats, params, nbr_in3, nbr_down, nbr_out3):
    feats = np.asarray(feats)
    nbr_in3 = np.asarray(nbr_in3)
    nbr_down = np.asarray(nbr_down)
    nbr_out3 = np.asarray(nbr_out3)

    if "nc" not in _NC_CACHE:
        _NC_CACHE["nc"] = build_module()
    nc = _NC_CACHE["nc"]

    in_maps = _prep_host(feats, params, nbr_in3, nbr_down, nbr_out3)
    res = run_bass_kernel_spmd(nc, in_maps, core_ids=list(range(NCORES)))
    out = np.concatenate(
        [res.results[c]["y"][:SH1] for c in range(NCORES)], axis=0
    ).astype(np.float32)
    return out
